# revision 10
# baseline (speedup 1.0000x reference)
"""Causal self-attention Trainium2 Bass kernel.

Problem: B=2, T=4096, C=512, H=8 heads, D=64.
  q = x@Wq.T, k = x@Wk.T, v = x@Wv.T  (per-head split)
  att = softmax(causal(q k^T / sqrt(D)));  y = att @ v;  out = y @ Wout.T

Sharding: 8 cores = 2 batches x 4 head-groups (2 heads each).
Each core computes, for its batch b and heads {2g, 2g+1}:
  - feature-major qT,kT,vT [128, T] via PE matmuls (float32r, 1 cyc/row)
  - per t1-chunk (512 wide): transposed scores ST[t2_block, t1] = kT^T qT,
    causal mask added on PE (identity x {0,-1e5} pattern), exp on ACT
    (scale=1/8), then yT_aug[65, t1] accumulation with v augmented by a
    ones column so row 64 = softmax denominator.
  - normalization via one PE broadcast matmul + DVE multiply, then the
    out-projection partial out = y_norm @ Wout[:, cols]^T  ->  [T, 512].
Host sums the 4 partial outputs per batch (row-parallel out projection).
"""

import os
import sys

import numpy as np

B, T, C = 2, 4096, 512
H, D = 8, 64
P = 128          # partitions / t2-block size
CH = 512         # t1 chunk width
NCH = T // CH    # 8 chunks
NTB = T // P     # 32 t-blocks
KC = C // P      # 4 contraction chunks for projections
NEG = -1.0e5     # causal mask additive value (pre-scale)

_COMPILED = None


def _import_concourse():
    try:
        import concourse.bass  # noqa: F401
    except ImportError:
        for p in ("/opt/trn_rl_repo", os.path.expanduser("~/.axon_site/_ro/trn_rl_repo")):
            if os.path.isdir(p) and p not in sys.path:
                sys.path.insert(0, p)
        import concourse.bass  # noqa: F401


def _build():
    """Build + compile the SPMD Bass program (same program on all 8 cores)."""
    _import_concourse()
    import concourse.bass as bass  # noqa: F401
    import concourse.tile as tile
    from concourse import bacc, mybir

    f32 = mybir.dt.float32
    f32r = mybir.dt.float32r
    EXP = mybir.ActivationFunctionType.Exp

    nc = bacc.Bacc("TRN2", target_bir_lowering=False, debug=False, num_devices=8)

    xT_d = nc.dram_tensor("xT", [C, T], f32r, kind="ExternalInput").ap()
    wq_d = nc.dram_tensor("wq", [P, C], f32r, kind="ExternalInput").ap()
    wk_d = nc.dram_tensor("wk", [P, C], f32r, kind="ExternalInput").ap()
    wv_d = nc.dram_tensor("wv", [P, C], f32r, kind="ExternalInput").ap()
    wo_d = nc.dram_tensor("wo", [P, C], f32r, kind="ExternalInput").ap()
    mk_d = nc.dram_tensor("mk", [P, 4 * CH], f32r, kind="ExternalInput").ap()
    sel_d = nc.dram_tensor("sel", [65, P], f32, kind="ExternalInput").ap()
    id_d = nc.dram_tensor("idm", [P, P], f32r, kind="ExternalInput").ap()
    out_d = nc.dram_tensor("out", [T, C], f32, kind="ExternalOutput").ap()

    import contextlib

    with tile.TileContext(nc) as tc, contextlib.ExitStack() as _pctx:
        # ---- persistent SBUF tensors
        persist = _pctx.enter_context(tc.tile_pool(name="persist", bufs=1))

        def ptile(shape, name, dt=f32):
            return persist.tile(shape, dt, name=name, tag=name)

        xT_sb = ptile([P, KC * T], "xT_sb", f32r)      # 8 MB
        wq_sb = ptile([P, C], "wq_sb", f32r)
        wk_sb = ptile([P, C], "wk_sb", f32r)
        wv_sb = ptile([P, C], "wv_sb", f32r)
        wo_sb = ptile([P, C], "wo_sb", f32r)
        mk_sb = ptile([P, 4 * CH], "mk_sb", f32r)
        sel_sb = ptile([65, P], "sel_sb")
        id_sb = ptile([P, P], "id_sb", f32r)
        qT_sb = ptile([P, T], "qT_sb", f32r)
        kT_sb = ptile([P, T], "kT_sb", f32r)
        va1_sb = ptile([P, NTB * 65], "va1_sb", f32r)
        va2_sb = ptile([P, NTB * 65], "va2_sb", f32r)
        yT_sb = ptile([P, T], "yT_sb")
        yTn_sb = ptile([P, T], "yTn_sb", f32r)

        # ---- input DMAs
        for k in range(KC):
            nc.sync.dma_start(xT_sb[:, T * k : T * (k + 1)], xT_d[P * k : P * (k + 1), :])
        nc.sync.dma_start(wq_sb[:], wq_d[:])
        nc.sync.dma_start(wk_sb[:], wk_d[:])
        nc.sync.dma_start(wv_sb[:], wv_d[:])
        nc.sync.dma_start(wo_sb[:], wo_d[:])
        nc.sync.dma_start(mk_sb[:], mk_d[:])
        nc.sync.dma_start(sel_sb[:], sel_d[:])
        nc.sync.dma_start(id_sb[:], id_d[:])
        ones_sb = ptile([P, NTB], "ones_sb")  # f32 staging for vaug ones cols
        nc.vector.memset(ones_sb[:], 1.0)
        ones_src = ones_sb[:].rearrange("p (b s) -> p b s", s=1)
        for va in (va1_sb, va2_sb):
            dst = va[:].rearrange("p (b s) -> p b s", s=65)[:, :, 64:65]
            nc.vector.tensor_copy(dst, ones_src)

        # ---- pools (PSUM: 2*2 + 2*1 + 2*1 = 8 banks)
        with contextlib.ExitStack() as ctx:
            ps_st = ctx.enter_context(tc.tile_pool(name="ps_st", bufs=2, space="PSUM"))
            ps_yt = ctx.enter_context(tc.tile_pool(name="ps_yt", bufs=2, space="PSUM"))
            ps_ms = ctx.enter_context(tc.tile_pool(name="ps_ms", bufs=2, space="PSUM"))
            sb_e = ctx.enter_context(tc.tile_pool(name="sb_e", bufs=3))
            sb_vt = ctx.enter_context(tc.tile_pool(name="sb_vt", bufs=2))
            sb_sm = ctx.enter_context(tc.tile_pool(name="sb_sm", bufs=2))
            sb_ob = ctx.enter_context(tc.tile_pool(name="sb_ob", bufs=3))

            for c in range(NCH):
                # ======== QKV for chunk c (feature-major)
                for w_sb, kind in ((wk_sb, "k"), (wv_sb, "v"), (wq_sb, "q")):
                    acc = ps_ms.tile([P, CH], f32, tag="ms", name=f"ps_{kind}{c}")
                    for k in range(KC):
                        nc.tensor.matmul(
                            acc[:],
                            (w_sb[:, P * k : P * (k + 1)]),
                            (xT_sb[:, T * k + CH * c : T * k + CH * (c + 1)]),
                            start=(k == 0),
                            stop=(k == KC - 1),
                        )
                    if kind == "k":
                        nc.vector.tensor_copy(kT_sb[:, CH * c : CH * (c + 1)], acc[:])
                    elif kind == "q":
                        nc.vector.tensor_copy(qT_sb[:, CH * c : CH * (c + 1)], acc[:])
                    else:
                        vt = sb_vt.tile([P, CH], f32r, tag="vt", name=f"vt{c}")
                        nc.vector.tensor_copy(vt[:], acc[:])
                        for b2 in range(4):
                            jb = 4 * c + b2
                            tr = ps_ms.tile([P, CH], f32r, tag="ms", name=f"tr{jb}")
                            nc.tensor.transpose(
                                tr[:, 0:P], vt[:, P * b2 : P * (b2 + 1)], id_sb[:]
                            )
                            nc.vector.tensor_copy(
                                va1_sb[:, 65 * jb : 65 * jb + 64], tr[:, 0:64]
                            )
                            nc.vector.tensor_copy(
                                va2_sb[:, 65 * jb : 65 * jb + 64], tr[:, 64:P]
                            )

                # ======== attention for chunk c
                njb = 4 * (c + 1)
                yt1 = ps_yt.tile([65, CH], f32, tag="yt", name=f"yt1_{c}")
                yt2 = ps_yt.tile([65, CH], f32, tag="yt", name=f"yt2_{c}")
                for j in range(njb):
                    stp = ps_st.tile([P, 2 * CH], f32, tag="st", name=f"st{c}_{j}")
                    diag = j >= 4 * c
                    for h in (0, 1):
                        sl = stp[:, CH * h : CH * (h + 1)]
                        nc.tensor.matmul(
                            sl,
                            (kT_sb[64 * h : 64 * (h + 1), P * j : P * (j + 1)]),
                            (qT_sb[64 * h : 64 * (h + 1), CH * c : CH * (c + 1)]),
                            start=True,
                            stop=not diag,
                        )
                        if diag:
                            r = j - 4 * c
                            nc.tensor.matmul(
                                sl,
                                (id_sb[:]),
                                (mk_sb[:, CH * r : CH * (r + 1)]),
                                start=False,
                                stop=True,
                            )
                    et = sb_e.tile([P, 2 * CH], f32r, tag="e", name=f"e{c}_{j}")
                    nc.scalar.activation(et[:], stp[:], EXP, scale=0.125)
                    for h, yt, va in ((0, yt1, va1_sb), (1, yt2, va2_sb)):
                        nc.tensor.matmul(
                            yt[:],
                            (va[:, 65 * j : 65 * j + 65]),
                            (et[:, CH * h : CH * (h + 1)]),
                            start=(j == 0),
                            stop=(j == njb - 1),
                        )

                # ======== chunk tail: normalize + out projection
                spair = sb_sm.tile([65, CH], f32, tag="sp", name=f"sp{c}")
                nc.vector.memset(spair[:], 1.0)
                nc.vector.tensor_copy(spair[0:1, :], yt1[64:65, :])
                nc.vector.tensor_copy(spair[64:65, :], yt2[64:65, :])
                rpair = sb_sm.tile([65, CH], f32, tag="rp", name=f"rp{c}")
                nc.vector.reciprocal(rpair[:], spair[:])
                nc.vector.tensor_copy(yT_sb[0:64, CH * c : CH * (c + 1)], yt1[0:64, :])
                nc.vector.tensor_copy(yT_sb[64:P, CH * c : CH * (c + 1)], yt2[0:64, :])
                rb = ps_ms.tile([P, CH], f32, tag="ms", name=f"rb{c}")
                # full-fp32 broadcast matmul (exact: sel is 0/1)
                nc.tensor.matmul(rb[:], sel_sb[:], rpair[:], start=True, stop=True)
                nc.vector.tensor_mul(
                    yTn_sb[:, CH * c : CH * (c + 1)],
                    yT_sb[:, CH * c : CH * (c + 1)],
                    rb[:],
                )
                for b2 in range(4):
                    tb = 4 * c + b2
                    op = ps_ms.tile([P, CH], f32, tag="ms", name=f"op{tb}")
                    nc.tensor.matmul(
                        op[:],
                        (yTn_sb[:, P * tb : P * (tb + 1)]),
                        (wo_sb[:]),
                        start=True,
                        stop=True,
                    )
                    ob = sb_ob.tile([P, CH], f32, tag="ob", name=f"ob{tb}")
                    nc.vector.tensor_copy(ob[:], op[:])
                    nc.sync.dma_start(out_d[P * tb : P * (tb + 1), :], ob[:])

    nc.compile()
    return nc


def _host_inputs(x, Wq, Wk, Wv, Wout):
    """Per-core input maps. Core c: batch b=c//4, head-group g=c%4."""
    x = np.asarray(x, dtype=np.float32)
    Wq = np.asarray(Wq, dtype=np.float32)
    Wk = np.asarray(Wk, dtype=np.float32)
    Wv = np.asarray(Wv, dtype=np.float32)
    Wout = np.asarray(Wout, dtype=np.float32)

    # causal additive mask patterns [128, 4*512]: pattern r valid iff col >= 128r + row
    col = np.arange(CH)[None, :]
    row = np.arange(P)[:, None]
    mk = np.concatenate(
        [np.where(col >= P * r + row, 0.0, NEG).astype(np.float32) for r in range(4)],
        axis=1,
    )
    sel = np.zeros((65, P), dtype=np.float32)
    sel[0, 0:64] = 1.0
    sel[64, 64:P] = 1.0
    idm = np.eye(P, dtype=np.float32)

    def arrange_w(Wc):  # Wc: [128 feat, 512 c] -> lhsT layout [p, (k m)]
        return np.concatenate(
            [np.ascontiguousarray(Wc[:, P * k : P * (k + 1)].T) for k in range(KC)],
            axis=1,
        )

    in_maps = []
    for core in range(8):
        b, g = core // 4, core % 4
        rows = slice(P * g, P * (g + 1))
        in_maps.append(
            {
                "xT": np.ascontiguousarray(x[b].T),
                "wq": arrange_w(Wq[rows]),
                "wk": arrange_w(Wk[rows]),
                "wv": arrange_w(Wv[rows]),
                "wo": np.ascontiguousarray(Wout[:, rows].T),
                "mk": mk,
                "sel": sel,
                "idm": idm,
            }
        )
    return in_maps


def _get_compiled():
    global _COMPILED
    if _COMPILED is None:
        _COMPILED = _build()
    return _COMPILED


def run_on_hw(x, Wq, Wk, Wv, Wout, trace=False):
    """Returns (full_output [B,T,C], exec_time_ns_or_None)."""
    _import_concourse()
    from concourse import bass_utils

    nc = _get_compiled()
    in_maps = _host_inputs(x, Wq, Wk, Wv, Wout)
    res = bass_utils.run_bass_kernel_spmd(
        nc, in_maps, list(range(8)), trace=trace
    )
    parts = [res.results[i]["out"] for i in range(8)]
    out = np.stack(
        [
            parts[0] + parts[1] + parts[2] + parts[3],
            parts[4] + parts[5] + parts[6] + parts[7],
        ]
    ).astype(np.float32)
    return out, res.exec_time_ns


def kernel(x, Wq, Wk, Wv, Wout):
    out, _ = run_on_hw(x, Wq, Wk, Wv, Wout, trace=False)
    return out


if __name__ == "__main__":
    # smoke test with random data (no reference)
    rng = np.random.default_rng(0)
    x = rng.standard_normal((B, T, C), dtype=np.float32)
    s = 1.0 / np.sqrt(C)
    ws = [rng.standard_normal((C, C), dtype=np.float32) * s for _ in range(4)]
    out = kernel(x, *ws)
    print("out", out.shape, out.dtype, np.abs(out).mean())


# revision 25
# speedup vs baseline: 1.6563x; 1.6563x over previous
"""Causal self-attention Trainium2 Bass kernel.

Problem: B=2, T=4096, C=512, H=8 heads, D=64.
  q = x@Wq.T, k = x@Wk.T, v = x@Wv.T  (per-head split)
  att = softmax(causal(q k^T / sqrt(D)));  y = att @ v;  out = y @ Wout.T

Sharding: 8 cores = 2 batches x 4 head-groups (2 heads each).
Each core computes, for its batch b and heads {2g, 2g+1}:
  - feature-major qT,kT,vT [128, T] via PE matmuls (float32r, 1 cyc/row)
  - per t1-chunk (512 wide): transposed scores ST[t2_block, t1] = kT^T qT,
    causal mask added on PE (identity x {0,-1e5} pattern), exp on ACT
    (scale=1/8), then yT_aug[65, t1] accumulation with v augmented by a
    ones column so row 64 = softmax denominator.
  - normalization via one PE broadcast matmul + DVE multiply, then the
    out-projection partial out = y_norm @ Wout[:, cols]^T  ->  [T, 512].
Host sums the 4 partial outputs per batch (row-parallel out projection).
"""

import os
import sys

import numpy as np

B, T, C = 2, 4096, 512
H, D = 8, 64
P = 128          # partitions / t2-block size
CH = 512         # t1 chunk width
NCH = T // CH    # 8 chunks
NTB = T // P     # 32 t-blocks
KC = C // P      # 4 contraction chunks for projections
NEG = -1.0e5     # causal mask additive value (pre-scale)

_COMPILED = None


def _import_concourse():
    try:
        import concourse.bass  # noqa: F401
    except ImportError:
        for p in ("/opt/trn_rl_repo", os.path.expanduser("~/.axon_site/_ro/trn_rl_repo")):
            if os.path.isdir(p) and p not in sys.path:
                sys.path.insert(0, p)
        import concourse.bass  # noqa: F401


def _build():
    """Build + compile the SPMD Bass program (same program on all 8 cores)."""
    _import_concourse()
    import concourse.bass as bass  # noqa: F401
    import concourse.tile as tile
    from concourse import bacc, mybir

    f32 = mybir.dt.float32
    f32r = mybir.dt.float32r
    EXP = mybir.ActivationFunctionType.Exp

    nc = bacc.Bacc("TRN2", target_bir_lowering=False, debug=False, num_devices=8)

    xT_d = nc.dram_tensor("xT", [C, T], f32r, kind="ExternalInput").ap()
    wq_d = nc.dram_tensor("wq", [P, C], f32r, kind="ExternalInput").ap()
    wk_d = nc.dram_tensor("wk", [P, C], f32r, kind="ExternalInput").ap()
    wv_d = nc.dram_tensor("wv", [P, C], f32r, kind="ExternalInput").ap()
    wo_d = nc.dram_tensor("wo", [P, C], f32r, kind="ExternalInput").ap()
    mk_d = nc.dram_tensor("mk", [P, 8 * CH], f32r, kind="ExternalInput").ap()
    sel_d = nc.dram_tensor("sel", [65, P], f32r, kind="ExternalInput").ap()
    id_d = nc.dram_tensor("idm", [P, P], f32r, kind="ExternalInput").ap()
    out_d = nc.dram_tensor("out", [T, C], f32, kind="ExternalOutput").ap()

    import contextlib

    with tile.TileContext(nc) as tc, contextlib.ExitStack() as _pctx:
        # ---- persistent SBUF tensors
        persist = _pctx.enter_context(tc.tile_pool(name="persist", bufs=1))

        def ptile(shape, name, dt=f32):
            return persist.tile(shape, dt, name=name, tag=name)

        xT_sb = ptile([P, KC * T], "xT_sb", f32r)      # 8 MB
        wq_sb = ptile([P, C], "wq_sb", f32r)
        wk_sb = ptile([P, C], "wk_sb", f32r)
        wv_sb = ptile([P, C], "wv_sb", f32r)
        wo_sb = ptile([P, C], "wo_sb", f32r)
        mk_sb = ptile([P, 8 * CH], "mk_sb", f32r)
        sel_sb = ptile([65, P], "sel_sb", f32r)
        id_sb = ptile([P, P], "id_sb", f32r)
        qT_sb = ptile([P, T], "qT_sb", f32r)
        kT_sb = ptile([P, T], "kT_sb", f32r)
        va1_sb = ptile([P, NTB * 65], "va1_sb", f32r)
        va2_sb = ptile([P, NTB * 65], "va2_sb", f32r)
        yT_sb = ptile([P, T], "yT_sb", f32r)

        # ---- input DMAs (weights first; xT split across sync+gpsimd queues)
        nc.sync.dma_start(wk_sb[:], wk_d[:])
        nc.sync.dma_start(wq_sb[:], wq_d[:])
        nc.sync.dma_start(wv_sb[:], wv_d[:])
        nc.sync.dma_start(wo_sb[:], wo_d[:])
        nc.sync.dma_start(sel_sb[:], sel_d[:])
        nc.sync.dma_start(id_sb[:], id_d[:])
        for k in range(KC):
            for hf in range(2):
                nc.sync.dma_start(
                    xT_sb[:, T * k + (T // 2) * hf : T * k + (T // 2) * (hf + 1)],
                    xT_d[P * k : P * (k + 1), (T // 2) * hf : (T // 2) * (hf + 1)],
                )
        nc.sync.dma_start(mk_sb[:], mk_d[:])
        ones_sb = ptile([P, NTB], "ones_sb")  # f32 staging for vaug ones cols
        nc.vector.memset(ones_sb[:], 1.0)
        warm_sb = ptile([P, NTB], "warm_sb")
        nc.scalar.activation(warm_sb[:], ones_sb[:], EXP, scale=0.125)
        ones_src = ones_sb[:].rearrange("p (b s) -> p b s", s=1)
        for va in (va1_sb, va2_sb):
            dst = va[:].rearrange("p (b s) -> p b s", s=65)[:, :, 64:65]
            nc.vector.tensor_copy(dst, ones_src)

        # ---- pools (PSUM: 2*2 + 2*1 + 2*1 = 8 banks)
        with contextlib.ExitStack() as ctx:
            ps_st = ctx.enter_context(tc.tile_pool(name="ps_st", bufs=2, space="PSUM"))
            ps_yt = ctx.enter_context(tc.tile_pool(name="ps_yt", bufs=2, space="PSUM"))
            ps_ms = ctx.enter_context(tc.tile_pool(name="ps_ms", bufs=2, space="PSUM"))
            sb_e = ctx.enter_context(tc.tile_pool(name="sb_e", bufs=5))
            sb_vt = ctx.enter_context(tc.tile_pool(name="sb_vt", bufs=2))
            sb_sm = ctx.enter_context(tc.tile_pool(name="sb_sm", bufs=2))
            sb_ob = ctx.enter_context(tc.tile_pool(name="sb_ob", bufs=4))

            def emit_qkv(c):
                for w_sb, kind in ((wk_sb, "k"), (wq_sb, "q"), (wv_sb, "v")):
                    acc = ps_ms.tile([P, CH], f32, tag="ms", name=f"ps_{kind}{c}")
                    for k in range(KC):
                        nc.tensor.matmul(
                            acc[:],
                            w_sb[:, P * k : P * (k + 1)],
                            xT_sb[:, T * k + CH * c : T * k + CH * (c + 1)],
                            start=(k == 0),
                            stop=(k == KC - 1),
                        )
                    if kind == "k":
                        nc.scalar.copy(kT_sb[:, CH * c : CH * (c + 1)], acc[:])
                    elif kind == "q":
                        nc.scalar.copy(qT_sb[:, CH * c : CH * (c + 1)], acc[:])
                    else:
                        vt = sb_vt.tile([P, CH], f32r, tag="vt", name=f"vt{c}")
                        nc.vector.tensor_copy(vt[:], acc[:])
                        for b2 in range(4):
                            jb = 4 * c + b2
                            tr = ps_ms.tile([P, CH], f32r, tag="ms", name=f"tr{jb}")
                            nc.tensor.transpose(
                                tr[:, 0:P], vt[:, P * b2 : P * (b2 + 1)], id_sb[:]
                            )
                            nc.vector.tensor_copy(
                                va1_sb[:, 65 * jb : 65 * jb + 64], tr[:, 0:64]
                            )
                            nc.vector.tensor_copy(
                                va2_sb[:, 65 * jb : 65 * jb + 64], tr[:, 64:P]
                            )

            def emit_attention(c, yts):
                """Software-pipelined: ST(j) emitted one step ahead of yT(j-1)."""
                njb = 4 * (c + 1)
                yt1, yt2 = yts

                def emit_st(j):
                    stp = ps_st.tile([P, 2 * CH], f32, tag="st", name=f"st{c}_{j}")
                    for h in (0, 1):
                        nc.tensor.matmul(
                            stp[:, CH * h : CH * (h + 1)],
                            kT_sb[64 * h : 64 * (h + 1), P * j : P * (j + 1)],
                            qT_sb[64 * h : 64 * (h + 1), CH * c : CH * (c + 1)],
                            start=True,
                            stop=True,
                        )
                    if j >= 4 * c:  # diagonal block: additive causal mask
                        r = j - 4 * c
                        nc.vector.tensor_add(
                            stp[:], stp[:], mk_sb[:, 2 * CH * r : 2 * CH * (r + 1)]
                        )
                    et = sb_e.tile([P, 2 * CH], f32r, tag="e", name=f"e{c}_{j}")
                    nc.scalar.activation(et[:], stp[:], EXP, scale=0.125)
                    return et

                def emit_yt(j, et):
                    for h, yt, va in ((0, yt1, va1_sb), (1, yt2, va2_sb)):
                        nc.tensor.matmul(
                            yt[:],
                            va[:, 65 * j : 65 * j + 65],
                            et[:, CH * h : CH * (h + 1)],
                            start=(j == 0),
                            stop=(j == njb - 1),
                        )

                pending = []
                for j in range(njb):
                    et = emit_st(j)
                    pending.append((j, et))
                    if len(pending) > 2:
                        emit_yt(*pending.pop(0))
                for p_ in pending:
                    emit_yt(*p_)

            def emit_tail(c, yts):
                yt1, yt2 = yts
                spair = sb_sm.tile([65, CH], f32, tag="sp", name=f"sp{c}")
                nc.vector.memset(spair[:], 1.0)
                nc.vector.tensor_copy(spair[0:1, :], yt1[64:65, :])
                nc.vector.tensor_copy(spair[64:65, :], yt2[64:65, :])
                rpair = sb_sm.tile([65, CH], f32r, tag="rp", name=f"rp{c}")
                with nc.allow_low_precision("f32r reciprocal for softmax norm"):
                    nc.vector.reciprocal(rpair[:], spair[:])
                nc.vector.tensor_copy(yT_sb[0:64, CH * c : CH * (c + 1)], yt1[0:64, :])
                nc.vector.tensor_copy(yT_sb[64:P, CH * c : CH * (c + 1)], yt2[0:64, :])
                rb = ps_ms.tile([P, CH], f32, tag="ms", name=f"rb{c}")
                # full-fp32 broadcast matmul (exact: sel is 0/1)
                nc.tensor.matmul(rb[:], sel_sb[:], rpair[:], start=True, stop=True)
                nc.vector.tensor_mul(
                    yT_sb[:, CH * c : CH * (c + 1)],
                    yT_sb[:, CH * c : CH * (c + 1)],
                    rb[:],
                )
                for b2 in range(4):
                    tb = 4 * c + b2
                    op = ps_ms.tile([P, CH], f32, tag="ms", name=f"op{tb}")
                    nc.tensor.matmul(
                        op[:],
                        yT_sb[:, P * tb : P * (tb + 1)],
                        wo_sb[:],
                        start=True,
                        stop=True,
                    )
                    ob = sb_ob.tile([P, CH], f32, tag="ob", name=f"ob{tb}")
                    nc.scalar.copy(ob[:], op[:])
                    nc.sync.dma_start(out_d[P * tb : P * (tb + 1), :], ob[:])

            prev = None  # (chunk, (yt1, yt2)) awaiting tail
            for c in range(NCH):
                emit_qkv(c)
                if prev is not None:
                    emit_tail(*prev)
                yts = (
                    ps_yt.tile([65, CH], f32, tag="yt", name=f"yt1_{c}"),
                    ps_yt.tile([65, CH], f32, tag="yt", name=f"yt2_{c}"),
                )
                emit_attention(c, yts)
                prev = (c, yts)
            emit_tail(*prev)

    nc.compile()
    return nc


def _host_inputs(x, Wq, Wk, Wv, Wout):
    """Per-core input maps. Core c: batch b=c//4, head-group g=c%4."""
    x = np.asarray(x, dtype=np.float32)
    Wq = np.asarray(Wq, dtype=np.float32)
    Wk = np.asarray(Wk, dtype=np.float32)
    Wv = np.asarray(Wv, dtype=np.float32)
    Wout = np.asarray(Wout, dtype=np.float32)

    # multiplicative causal masks [128, 8*512]: pattern r (doubled for the two
    # heads of a strip) valid iff col >= 128r + row
    col = np.arange(CH)[None, :]
    row = np.arange(P)[:, None]
    pats = [np.where(col >= P * r + row, 0.0, NEG).astype(np.float32) for r in range(4)]
    mk = np.concatenate([np.tile(p, (1, 2)) for p in pats], axis=1)
    sel = np.zeros((65, P), dtype=np.float32)
    sel[0, 0:64] = 1.0
    sel[64, 64:P] = 1.0
    idm = np.eye(P, dtype=np.float32)

    def arrange_w(Wc):  # Wc: [128 feat, 512 c] -> lhsT layout [p, (k m)]
        return np.concatenate(
            [np.ascontiguousarray(Wc[:, P * k : P * (k + 1)].T) for k in range(KC)],
            axis=1,
        )

    in_maps = []
    for core in range(8):
        b, g = core // 4, core % 4
        rows = slice(P * g, P * (g + 1))
        in_maps.append(
            {
                "xT": np.ascontiguousarray(x[b].T),
                "wq": arrange_w(Wq[rows]),
                "wk": arrange_w(Wk[rows]),
                "wv": arrange_w(Wv[rows]),
                "wo": np.ascontiguousarray(Wout[:, rows].T),
                "mk": mk,
                "sel": sel,
                "idm": idm,
            }
        )
    return in_maps


def _get_compiled():
    global _COMPILED
    if _COMPILED is None:
        _COMPILED = _build()
    return _COMPILED


def run_on_hw(x, Wq, Wk, Wv, Wout, trace=False):
    """Returns (full_output [B,T,C], exec_time_ns_or_None)."""
    _import_concourse()
    from concourse import bass_utils

    nc = _get_compiled()
    in_maps = _host_inputs(x, Wq, Wk, Wv, Wout)
    res = bass_utils.run_bass_kernel_spmd(
        nc, in_maps, list(range(8)), trace=trace
    )
    global LAST_RESULT
    LAST_RESULT = res
    parts = [res.results[i]["out"] for i in range(8)]
    out = np.stack(
        [
            parts[0] + parts[1] + parts[2] + parts[3],
            parts[4] + parts[5] + parts[6] + parts[7],
        ]
    ).astype(np.float32)
    return out, res.exec_time_ns


def kernel(x, Wq, Wk, Wv, Wout):
    out, _ = run_on_hw(x, Wq, Wk, Wv, Wout, trace=False)
    return out


if __name__ == "__main__":
    # smoke test with random data (no reference)
    rng = np.random.default_rng(0)
    x = rng.standard_normal((B, T, C), dtype=np.float32)
    s = 1.0 / np.sqrt(C)
    ws = [rng.standard_normal((C, C), dtype=np.float32) * s for _ in range(4)]
    out = kernel(x, *ws)
    print("out", out.shape, out.dtype, np.abs(out).mean())


# revision 27
# speedup vs baseline: 1.6616x; 1.0032x over previous
"""Causal self-attention Trainium2 Bass kernel.

Problem: B=2, T=4096, C=512, H=8 heads, D=64.
  q = x@Wq.T, k = x@Wk.T, v = x@Wv.T  (per-head split)
  att = softmax(causal(q k^T / sqrt(D)));  y = att @ v;  out = y @ Wout.T

Sharding: 8 cores = 2 batches x 4 head-groups (2 heads each).
Each core computes, for its batch b and heads {2g, 2g+1}:
  - feature-major qT,kT,vT [128, T] via PE matmuls (float32r, 1 cyc/row)
  - per t1-chunk (512 wide): transposed scores ST[t2_block, t1] = kT^T qT,
    causal mask added on PE (identity x {0,-1e5} pattern), exp on ACT
    (scale=1/8), then yT_aug[65, t1] accumulation with v augmented by a
    ones column so row 64 = softmax denominator.
  - normalization via one PE broadcast matmul + DVE multiply, then the
    out-projection partial out = y_norm @ Wout[:, cols]^T  ->  [T, 512].
Host sums the 4 partial outputs per batch (row-parallel out projection).
"""

import os
import sys

import numpy as np

B, T, C = 2, 4096, 512
H, D = 8, 64
P = 128          # partitions / t2-block size
CH = 512         # t1 chunk width
NCH = T // CH    # 8 chunks
NTB = T // P     # 32 t-blocks
KC = C // P      # 4 contraction chunks for projections
NEG = -1.0e5     # causal mask additive value (pre-scale)

_COMPILED = None


def _import_concourse():
    try:
        import concourse.bass  # noqa: F401
    except ImportError:
        for p in ("/opt/trn_rl_repo", os.path.expanduser("~/.axon_site/_ro/trn_rl_repo")):
            if os.path.isdir(p) and p not in sys.path:
                sys.path.insert(0, p)
        import concourse.bass  # noqa: F401


def _build():
    """Build + compile the SPMD Bass program (same program on all 8 cores)."""
    _import_concourse()
    import concourse.bass as bass  # noqa: F401
    import concourse.tile as tile
    from concourse import bacc, mybir

    f32 = mybir.dt.float32
    f32r = mybir.dt.float32r
    EXP = mybir.ActivationFunctionType.Exp

    nc = bacc.Bacc("TRN2", target_bir_lowering=False, debug=False, num_devices=8)

    xT_d = nc.dram_tensor("xT", [C, T], f32r, kind="ExternalInput").ap()
    wq_d = nc.dram_tensor("wq", [P, C], f32r, kind="ExternalInput").ap()
    wk_d = nc.dram_tensor("wk", [P, C], f32r, kind="ExternalInput").ap()
    wv_d = nc.dram_tensor("wv", [P, C], f32r, kind="ExternalInput").ap()
    wo_d = nc.dram_tensor("wo", [P, C], f32r, kind="ExternalInput").ap()
    mk_d = nc.dram_tensor("mk", [P, 8 * CH], f32r, kind="ExternalInput").ap()
    sel_d = nc.dram_tensor("sel", [65, P], f32r, kind="ExternalInput").ap()
    id_d = nc.dram_tensor("idm", [P, P], f32r, kind="ExternalInput").ap()
    out_d = nc.dram_tensor("out", [T, C], f32, kind="ExternalOutput").ap()

    import contextlib

    with tile.TileContext(nc) as tc, contextlib.ExitStack() as _pctx:
        # ---- persistent SBUF tensors
        persist = _pctx.enter_context(tc.tile_pool(name="persist", bufs=1))

        def ptile(shape, name, dt=f32):
            return persist.tile(shape, dt, name=name, tag=name)

        xT_sb = ptile([P, KC * T], "xT_sb", f32r)      # 8 MB
        wq_sb = ptile([P, C], "wq_sb", f32r)
        wk_sb = ptile([P, C], "wk_sb", f32r)
        wv_sb = ptile([P, C], "wv_sb", f32r)
        wo_sb = ptile([P, C], "wo_sb", f32r)
        mk_sb = ptile([P, 8 * CH], "mk_sb", f32r)
        sel_sb = ptile([65, P], "sel_sb", f32r)
        id_sb = ptile([P, P], "id_sb", f32r)
        qT_sb = ptile([P, T], "qT_sb", f32r)
        kT_sb = ptile([P, T], "kT_sb", f32r)
        va1_sb = ptile([P, NTB * 65], "va1_sb", f32r)
        va2_sb = ptile([P, NTB * 65], "va2_sb", f32r)
        yT_sb = ptile([P, T], "yT_sb", f32r)

        # ---- input DMAs (weights first; xT split across sync+gpsimd queues)
        nc.sync.dma_start(wk_sb[:], wk_d[:])
        nc.sync.dma_start(wq_sb[:], wq_d[:])
        nc.sync.dma_start(wv_sb[:], wv_d[:])
        nc.sync.dma_start(wo_sb[:], wo_d[:])
        nc.sync.dma_start(sel_sb[:], sel_d[:])
        nc.sync.dma_start(id_sb[:], id_d[:])
        for k in range(KC):
            for hf in range(2):
                nc.sync.dma_start(
                    xT_sb[:, T * k + (T // 2) * hf : T * k + (T // 2) * (hf + 1)],
                    xT_d[P * k : P * (k + 1), (T // 2) * hf : (T // 2) * (hf + 1)],
                )
        nc.sync.dma_start(mk_sb[:], mk_d[:])
        ones_sb = ptile([P, NTB], "ones_sb")  # f32 staging for vaug ones cols
        nc.vector.memset(ones_sb[:], 1.0)
        warm_sb = ptile([P, NTB], "warm_sb")
        nc.scalar.activation(warm_sb[:], ones_sb[:], EXP, scale=0.125)
        ones_src = ones_sb[:].rearrange("p (b s) -> p b s", s=1)
        for va in (va1_sb, va2_sb):
            dst = va[:].rearrange("p (b s) -> p b s", s=65)[:, :, 64:65]
            nc.vector.tensor_copy(dst, ones_src)

        # ---- pools (PSUM: 2*2 + 2*1 + 2*1 = 8 banks)
        with contextlib.ExitStack() as ctx:
            ps_st = ctx.enter_context(tc.tile_pool(name="ps_st", bufs=2, space="PSUM"))
            ps_yt = ctx.enter_context(tc.tile_pool(name="ps_yt", bufs=2, space="PSUM"))
            ps_ms = ctx.enter_context(tc.tile_pool(name="ps_ms", bufs=2, space="PSUM"))
            sb_e = ctx.enter_context(tc.tile_pool(name="sb_e", bufs=5))
            sb_vt = ctx.enter_context(tc.tile_pool(name="sb_vt", bufs=2))
            sb_sm = ctx.enter_context(tc.tile_pool(name="sb_sm", bufs=2))
            sb_ob = ctx.enter_context(tc.tile_pool(name="sb_ob", bufs=4))

            def emit_qkv(c):
                for w_sb, kind in ((wk_sb, "k"), (wq_sb, "q"), (wv_sb, "v")):
                    acc = ps_ms.tile([P, CH], f32, tag="ms", name=f"ps_{kind}{c}")
                    for k in range(KC):
                        nc.tensor.matmul(
                            acc[:],
                            w_sb[:, P * k : P * (k + 1)],
                            xT_sb[:, T * k + CH * c : T * k + CH * (c + 1)],
                            start=(k == 0),
                            stop=(k == KC - 1),
                        )
                    if kind == "k":
                        nc.scalar.copy(kT_sb[:, CH * c : CH * (c + 1)], acc[:])
                    elif kind == "q":
                        nc.scalar.copy(qT_sb[:, CH * c : CH * (c + 1)], acc[:])
                    else:
                        vt = sb_vt.tile([P, CH], f32r, tag="vt", name=f"vt{c}")
                        nc.vector.tensor_copy(vt[:], acc[:])
                        for b2 in range(4):
                            jb = 4 * c + b2
                            tr = ps_ms.tile([P, CH], f32r, tag="ms", name=f"tr{jb}")
                            nc.tensor.transpose(
                                tr[:, 0:P], vt[:, P * b2 : P * (b2 + 1)], id_sb[:]
                            )
                            nc.vector.tensor_copy(
                                va1_sb[:, 65 * jb : 65 * jb + 64], tr[:, 0:64]
                            )
                            nc.vector.tensor_copy(
                                va2_sb[:, 65 * jb : 65 * jb + 64], tr[:, 64:P]
                            )

            def emit_attention(c, yts):
                """Software-pipelined: ST(j) emitted one step ahead of yT(j-1)."""
                njb = 4 * (c + 1)
                yt1, yt2 = yts

                def emit_st(j):
                    stp = ps_st.tile([P, 2 * CH], f32, tag="st", name=f"st{c}_{j}")
                    for h in (0, 1):
                        nc.tensor.matmul(
                            stp[:, CH * h : CH * (h + 1)],
                            kT_sb[64 * h : 64 * (h + 1), P * j : P * (j + 1)],
                            qT_sb[64 * h : 64 * (h + 1), CH * c : CH * (c + 1)],
                            start=True,
                            stop=True,
                        )
                    if j >= 4 * c:  # diagonal block: additive causal mask
                        r = j - 4 * c
                        nc.vector.tensor_add(
                            stp[:], stp[:], mk_sb[:, 2 * CH * r : 2 * CH * (r + 1)]
                        )
                    et = sb_e.tile([P, 2 * CH], f32r, tag="e", name=f"e{c}_{j}")
                    nc.scalar.activation(et[:], stp[:], EXP, scale=0.125)
                    return et

                def emit_yt(j, et):
                    for h, yt, va in ((0, yt1, va1_sb), (1, yt2, va2_sb)):
                        nc.tensor.matmul(
                            yt[:],
                            va[:, 65 * j : 65 * j + 65],
                            et[:, CH * h : CH * (h + 1)],
                            start=(j == 0),
                            stop=(j == njb - 1),
                        )

                pending = []
                for j in range(njb):
                    et = emit_st(j)
                    pending.append((j, et))
                    if len(pending) > 2:
                        emit_yt(*pending.pop(0))
                for p_ in pending:
                    emit_yt(*p_)

            def emit_tail(c, yts):
                yt1, yt2 = yts
                spair = sb_sm.tile([65, CH], f32, tag="sp", name=f"sp{c}")
                nc.vector.memset(spair[:], 1.0)
                nc.vector.tensor_copy(spair[0:1, :], yt1[64:65, :])
                nc.vector.tensor_copy(spair[64:65, :], yt2[64:65, :])
                rpair = sb_sm.tile([65, CH], f32r, tag="rp", name=f"rp{c}")
                with nc.allow_low_precision("f32r reciprocal for softmax norm"):
                    nc.vector.reciprocal(rpair[:], spair[:])
                nc.vector.tensor_copy(yT_sb[0:64, CH * c : CH * (c + 1)], yt1[0:64, :])
                nc.vector.tensor_copy(yT_sb[64:P, CH * c : CH * (c + 1)], yt2[0:64, :])
                rb = ps_ms.tile([P, CH], f32, tag="ms", name=f"rb{c}")
                # full-fp32 broadcast matmul (exact: sel is 0/1)
                nc.tensor.matmul(rb[:], sel_sb[:], rpair[:], start=True, stop=True)
                nc.vector.tensor_mul(
                    yT_sb[:, CH * c : CH * (c + 1)],
                    yT_sb[:, CH * c : CH * (c + 1)],
                    rb[:],
                )
                for b2 in range(4):
                    tb = 4 * c + b2
                    op = ps_ms.tile([P, CH], f32, tag="ms", name=f"op{tb}")
                    nc.tensor.matmul(
                        op[:],
                        yT_sb[:, P * tb : P * (tb + 1)],
                        wo_sb[:],
                        start=True,
                        stop=True,
                    )
                    ob = sb_ob.tile([P, CH], f32, tag="ob", name=f"ob{tb}")
                    nc.scalar.copy(ob[:], op[:])
                    nc.sync.dma_start(out_d[P * tb : P * (tb + 1), :], ob[:])

            prev = None  # (chunk, (yt1, yt2)) awaiting tail
            for c in range(NCH):
                emit_qkv(c)
                if prev is not None:
                    emit_tail(*prev)
                yts = (
                    ps_yt.tile([65, CH], f32, tag="yt", name=f"yt1_{c}"),
                    ps_yt.tile([65, CH], f32, tag="yt", name=f"yt2_{c}"),
                )
                emit_attention(c, yts)
                prev = (c, yts)
            emit_tail(*prev)

    nc.compile()
    return nc


def _host_inputs(x, Wq, Wk, Wv, Wout):
    """Per-core input maps. Core c: batch b=c//4, head-group g=c%4."""
    x = np.asarray(x, dtype=np.float32)
    Wq = np.asarray(Wq, dtype=np.float32)
    Wk = np.asarray(Wk, dtype=np.float32)
    Wv = np.asarray(Wv, dtype=np.float32)
    Wout = np.asarray(Wout, dtype=np.float32)

    # multiplicative causal masks [128, 8*512]: pattern r (doubled for the two
    # heads of a strip) valid iff col >= 128r + row
    col = np.arange(CH)[None, :]
    row = np.arange(P)[:, None]
    pats = [np.where(col >= P * r + row, 0.0, NEG).astype(np.float32) for r in range(4)]
    mk = np.concatenate([np.tile(p, (1, 2)) for p in pats], axis=1)
    sel = np.zeros((65, P), dtype=np.float32)
    sel[0, 0:64] = 1.0
    sel[64, 64:P] = 1.0
    idm = np.eye(P, dtype=np.float32)

    def arrange_w(Wc):  # Wc: [128 feat, 512 c] -> lhsT layout [p, (k m)]
        return np.concatenate(
            [np.ascontiguousarray(Wc[:, P * k : P * (k + 1)].T) for k in range(KC)],
            axis=1,
        )

    in_maps = []
    for core in range(8):
        b, g = core // 4, core % 4
        rows = slice(P * g, P * (g + 1))
        in_maps.append(
            {
                "xT": np.ascontiguousarray(x[b].T),
                "wq": arrange_w(Wq[rows]),
                "wk": arrange_w(Wk[rows]),
                "wv": arrange_w(Wv[rows]),
                "wo": np.ascontiguousarray(Wout[:, rows].T),
                "mk": mk,
                "sel": sel,
                "idm": idm,
            }
        )
    return in_maps


def _get_compiled():
    global _COMPILED
    if _COMPILED is None:
        _COMPILED = _build()
    return _COMPILED


def run_on_hw(x, Wq, Wk, Wv, Wout, trace=False):
    """Returns (full_output [B,T,C], exec_time_ns_or_None)."""
    _import_concourse()
    from concourse import bass_utils

    nc = _get_compiled()
    in_maps = _host_inputs(x, Wq, Wk, Wv, Wout)
    res = bass_utils.run_bass_kernel_spmd(
        nc, in_maps, list(range(8)), trace=trace
    )
    global LAST_RESULT
    LAST_RESULT = res
    parts = [res.results[i]["out"] for i in range(8)]
    out = np.stack(
        [
            parts[0] + parts[1] + parts[2] + parts[3],
            parts[4] + parts[5] + parts[6] + parts[7],
        ]
    ).astype(np.float32)
    return out, res.exec_time_ns


def kernel(x, Wq, Wk, Wv, Wout):
    out, _ = run_on_hw(x, Wq, Wk, Wv, Wout, trace=False)
    return out


if __name__ == "__main__":
    # smoke test with random data (no reference)
    rng = np.random.default_rng(0)
    x = rng.standard_normal((B, T, C), dtype=np.float32)
    s = 1.0 / np.sqrt(C)
    ws = [rng.standard_normal((C, C), dtype=np.float32) * s for _ in range(4)]
    out = kernel(x, *ws)
    print("out", out.shape, out.dtype, np.abs(out).mean())
